# revision 6
# baseline (speedup 1.0000x reference)
"""GQA prefill attention (B=2, S=2048, D=2048, H=32, KV=8, HD=64) on 8 trn2 cores.

Sharding: batch x kv-pair. Core c = (b = c//4, g = c%4) owns batch b,
q-heads [8g, 8g+8) and kv-heads {2g, 2g+1}; computes its partial of
out_b = attn_out @ wo_chunk; host sums the 4 partials per batch.

Device kernel (per core, bf16 matmuls / fp32 PSUM), per 512-seq stripe:
  proj: psum[128, 512] = wqkv-chunk^T @ xT-chunk (6 chunks: 4 q-head-pairs,
        K=[k0|k1], V=[v0|v1]); RoPE via pair-swap permutation matmul +
        cos/sin tables; V transposed to [s, dh] on PE; K duplicated on both
        partition halves for row-tiled scores.
  attn (per head-pair, per 256-q block): two K=64 score matmuls run
        concurrently in the two PE row-halves (tile_position (0,0)/(64,0)),
        sharing one psum bank [128k, 256q_e | 256q_o]; exp on ACT; partially
        masked blocks multiplied by exp(mask^T) tiles; one PV matmul
        lhsT=[V|1] rhs=P gives OT[dh|rowsum, 512] accumulated over k-blocks.
  norm: reciprocal of rowsum row, broadcast via K=1 ones-matmul, DVE mul.
  wo:   out[q-128, 512] += OT-chunks^T @ wo-chunk, fp32 DMA out.
"""

import os
import sys

import numpy as np
import ml_dtypes

BF16 = ml_dtypes.bfloat16

B, S, D, H, KV, HD = 2, 2048, 2048, 32, 8, 64
NCORES = 8
DC = D // 128       # 16 contraction chunks
NQI = S // 256      # 8 q-blocks of 256
KT_TILES = S // 128  # 16 k-blocks of 128


def _host_prepare(x, wq, wk, wv, wo, freqs, mask):
    """Build per-core device inputs + the mask block schedule."""
    # RoPE tables in the [dh-on-partitions, s] layout used by QT/KT.
    # Two 64-row head copies stacked (head pairs live on 128 partitions).
    # rope: out[2j]   = t[2j] cos - t[2j+1] sin
    #       out[2j+1] = t[2j] sin + t[2j+1] cos
    # with swap(t)[d] = t[d^1]:  out[d] = t[d]*cos[d] + swap(t)[d]*sgn(d)*sin[d]
    c64 = np.cos(freqs.T).repeat(2, axis=0).astype(np.float64)  # [64, S]
    s64 = np.sin(freqs.T).repeat(2, axis=0).astype(np.float64)
    sgn = np.where(np.arange(HD) % 2 == 0, -1.0, 1.0)[:, None]
    cos_t = np.concatenate([c64, c64], axis=0).astype(BF16)           # [128, S]
    sin_t = np.concatenate([s64 * sgn, s64 * sgn], axis=0).astype(BF16)

    # Mask block schedule at [128 k x 256 q] granularity (same for all b, h).
    # Block (qi, kt): full (mask all zero), skip (all <= -30), or masked
    # (multiply exp'd P by exp(mask^T) tile, stored duplicated to 512 cols
    # so one tile covers both heads of a pair).
    mt_tiles = []   # unique [128, 512] multiplier tiles
    mt_keys = {}
    sched = []      # per qi: list of (kt, mtile_idx | None)
    for qi in range(NQI):
        lst = []
        for kt in range(KT_TILES):
            blk = mask[qi * 256:(qi + 1) * 256, kt * 128:(kt + 1) * 128]  # [q, k]
            if np.all(blk <= -30.0):
                continue
            if np.all(blk == 0.0):
                lst.append((kt, None))
                continue
            t256 = np.exp(blk.T.astype(np.float64)).astype(BF16)  # [128 k, 256 q]
            tile_np = np.concatenate([t256, t256], axis=1)        # [128, 512]
            key = tile_np.tobytes()
            if key not in mt_keys:
                mt_keys[key] = len(mt_tiles)
                mt_tiles.append(tile_np)
            lst.append((kt, mt_keys[key]))
        if not lst:  # keep softmax denominators well-defined
            lst = [(kt, None) for kt in range(KT_TILES)]
        sched.append(lst)
    if not mt_tiles:
        mt_tiles.append(np.ones((128, 512), dtype=BF16))
    mt = np.stack(mt_tiles)  # [U, 128, 512]

    per_core = []
    for c in range(NCORES):
        b, g = c // 4, c % 4
        # xT[d, s] bf16 for this core's batch
        xT = np.ascontiguousarray(x[b].T).astype(BF16)
        # packed projection weights: 4 q-head-pairs (512), K pair (128), V pair (128)
        wqkv = np.concatenate(
            [wq[:, g * 8 * HD:(g + 1) * 8 * HD],
             wk[:, g * 2 * HD:(g + 1) * 2 * HD],
             wv[:, g * 2 * HD:(g + 1) * 2 * HD]], axis=1)
        per_core.append({
            "xT": xT,
            "wqkv": np.ascontiguousarray(wqkv).astype(BF16),
            "wo": np.ascontiguousarray(wo[g * 8 * HD:(g + 1) * 8 * HD, :]).astype(BF16),
            "cos": cos_t,
            "sin": sin_t,
            "mt": mt,
        })
    return per_core, sched, mt.shape[0]


def _build_program(sched, U):
    import concourse.bass as bass
    import concourse.mybir as mybir
    import concourse.tile as tile
    from concourse import bacc
    from concourse.tile_rust import add_dep_helper

    dt = mybir.dt
    bf, f32 = dt.bfloat16, dt.float32
    AF = mybir.ActivationFunctionType

    nc = bacc.Bacc("TRN2", target_bir_lowering=False, debug=False,
                   num_devices=NCORES)

    xT = nc.dram_tensor("xT", [D, S], bf, kind="ExternalInput")
    wqkv = nc.dram_tensor("wqkv", [D, 768], bf, kind="ExternalInput")
    wo = nc.dram_tensor("wo", [512, D], bf, kind="ExternalInput")
    cos = nc.dram_tensor("cos", [128, S], bf, kind="ExternalInput")
    sin = nc.dram_tensor("sin", [128, S], bf, kind="ExternalInput")
    mt = nc.dram_tensor("mt", [U, 128, 512], bf, kind="ExternalInput")
    out = nc.dram_tensor("out", [S, D], f32, kind="ExternalOutput")

    # pair-swap permutation (block-diag over the two stacked 64-row heads)
    perm_np = np.zeros((128, 128), dtype=BF16)
    for d in range(128):
        perm_np[d ^ 1, d] = 1
    perm_dram = nc.inline_tensor(np.ascontiguousarray(perm_np), name="perm")
    ident_dram = nc.inline_tensor(np.eye(128, dtype=BF16), name="ident")

    with tile.TileContext(nc) as tc:
        with (
            tc.tile_pool(name="const", bufs=1) as cp,
            tc.tile_pool(name="xt", bufs=2) as xp,
            tc.tile_pool(name="raw", bufs=3) as rawp,
            tc.tile_pool(name="rtmp", bufs=2) as rtp,
            tc.tile_pool(name="pt", bufs=3) as ptp,
            tc.tile_pool(name="ot", bufs=2) as otp,
            tc.tile_pool(name="bc", bufs=2) as bcp,
            tc.tile_pool(name="ri", bufs=2) as rip,
            tc.tile_pool(name="wsb", bufs=3) as wsp,
            tc.tile_pool(name="ps_p", bufs=2, space="PSUM") as pp,
            tc.tile_pool(name="ps_m", bufs=2, space="PSUM") as pm,
            tc.tile_pool(name="ps_s", bufs=2, space="PSUM") as pss,
            tc.tile_pool(name="ps_o", bufs=2, space="PSUM") as pso,
        ):
            wqkv_sb = cp.tile([128, DC, 768], bf)
            nc.sync.dma_start(wqkv_sb[:], wqkv.ap().rearrange("(c p) m -> p c m", p=128))
            cos_sb = cp.tile([128, S], bf)
            nc.sync.dma_start(cos_sb[:], cos.ap())
            sin_sb = cp.tile([128, S], bf)
            nc.sync.dma_start(sin_sb[:], sin.ap())
            perm_sb = cp.tile([128, 128], bf)
            nc.sync.dma_start(perm_sb[:], perm_dram.ap())
            ident_sb = cp.tile([128, 128], bf)
            nc.sync.dma_start(ident_sb[:], ident_dram.ap())
            mt_sb = cp.tile([128, U, 512], bf)
            nc.sync.dma_start(mt_sb[:], mt.ap().rearrange("u p q -> p u q"))
            wo_sb = cp.tile([128, 4, D], bf)
            nc.sync.dma_start(wo_sb[:], wo.ap().rearrange("(g p) n -> p g n", p=128))

            qt_sb = cp.tile([128, 4, S], bf)    # [pair-dh (even|odd), pair, s]
            kt_sb = cp.tile([128, 2, S], bf)    # [dh (dup halves), kv, s]
            vone_sb = cp.tile([128, KT_TILES, 130], bf)  # [s%128, kt, v0|1|v1|1]
            nc.vector.memset(vone_sb[:, :, 64:65], 1.0)
            nc.vector.memset(vone_sb[:, :, 129:130], 1.0)
            ones_sb = cp.tile([1, 64], bf)
            nc.vector.memset(ones_sb[:], 1.0)

            for st in range(4):
                s0 = st * 512
                # ---- projections + rope for this stripe ----
                xbig = xp.tile([128, DC, 512], bf, tag="x")
                xr = xT.ap().rearrange("(c p) s -> p c s", p=128)
                for sub in range(4):
                    nc.sync.dma_start(xbig[:, 4 * sub:4 * sub + 4, :],
                                      xr[:, 4 * sub:4 * sub + 4, s0:s0 + 512])
                for m in range(6):  # 0-3 q pairs, 4 = K pair, 5 = V pair
                    ps = pp.tile([128, 512], f32, tag="proj")
                    for dc in range(DC):
                        nc.tensor.matmul(ps[:], lhsT=wqkv_sb[:, dc, m * 128:(m + 1) * 128],
                                         rhs=xbig[:, dc, :],
                                         start=(dc == 0), stop=(dc == DC - 1))
                    raw = rawp.tile([128, 512], bf, tag="raw")
                    nc.vector.tensor_copy(raw[:], ps[:])
                    if m < 4:
                        sw = pm.tile([128, 512], f32, tag="misc")
                        nc.tensor.matmul(sw[:], lhsT=perm_sb[:], rhs=raw[:],
                                         start=True, stop=True)
                        tsin = rtp.tile([128, 512], bf, tag="tsin")
                        nc.vector.tensor_mul(tsin[:], sw[:], sin_sb[:, s0:s0 + 512])
                        tcos = rtp.tile([128, 512], bf, tag="tcos")
                        nc.vector.tensor_mul(tcos[:], raw[:], cos_sb[:, s0:s0 + 512])
                        nc.vector.tensor_add(qt_sb[:, m, s0:s0 + 512],
                                             tsin[:], tcos[:])
                    elif m == 4:
                        sw = pm.tile([128, 512], f32, tag="misc")
                        nc.tensor.matmul(sw[:], lhsT=perm_sb[:], rhs=raw[:],
                                         start=True, stop=True)
                        tsin = rtp.tile([128, 512], bf, tag="tsin")
                        nc.vector.tensor_mul(tsin[:], sw[:], sin_sb[:, s0:s0 + 512])
                        tcos = rtp.tile([128, 512], bf, tag="tcos")
                        nc.vector.tensor_mul(tcos[:], raw[:], cos_sb[:, s0:s0 + 512])
                        # k0 rotated = rows 0:64, k1 = rows 64:128; duplicate
                        # each onto both partition halves for row-tiled scores
                        krot = rtp.tile([128, 512], bf, tag="krot")
                        nc.vector.tensor_add(krot[:], tsin[:], tcos[:])
                        nc.vector.tensor_copy(kt_sb[0:64, 0, s0:s0 + 512],
                                              krot[0:64, :])
                        nc.vector.tensor_copy(kt_sb[64:128, 0, s0:s0 + 512],
                                              krot[0:64, :])
                        nc.vector.tensor_copy(kt_sb[0:64, 1, s0:s0 + 512],
                                              krot[64:128, :])
                        nc.vector.tensor_copy(kt_sb[64:128, 1, s0:s0 + 512],
                                              krot[64:128, :])
                    else:
                        for j in range(4):
                            mv = pm.tile([128, 512], f32, tag="misc")
                            vt = mv[:, 0:64].bitcast(bf)  # [128, 128] bf16 view
                            nc.tensor.transpose(vt, raw[:, j * 128:(j + 1) * 128],
                                                ident_sb[:])
                            kt_idx = 4 * st + j
                            nc.vector.tensor_copy(vone_sb[:, kt_idx, 0:64],
                                                  vt[:, 0:64])
                            nc.vector.tensor_copy(vone_sb[:, kt_idx, 65:129],
                                                  vt[:, 64:128])

                # ---- attention for q-blocks of this stripe ----
                ot_t = otp.tile([128, 4, 512], bf, tag="ot_t")  # [dh e|o, pair, q]
                for pair in range(4):
                    kv = pair // 2
                    for qi in (2 * st, 2 * st + 1):
                        q0 = qi * 256
                        kts = sched[qi]
                        otps = pso.tile([65, 512], f32, tag="otp")
                        for idx, (kt, mi) in enumerate(kts):
                            sp = pss.tile([128, 512], f32, tag="sc")
                            ma = nc.tensor.matmul(
                                sp[:, 0:256],
                                lhsT=kt_sb[0:64, kv, kt * 128:(kt + 1) * 128],
                                rhs=qt_sb[0:64, pair, q0:q0 + 256],
                                start=True, stop=False)
                            mb = nc.tensor.matmul(
                                sp[:, 256:512],
                                lhsT=kt_sb[64:128, kv, kt * 128:(kt + 1) * 128],
                                rhs=qt_sb[64:128, pair, q0:q0 + 256],
                                start=False, stop=True)
                            add_dep_helper(mb.ins, ma.ins, sync=False,
                                           reason="scores bank clear order")
                            pt = ptp.tile([128, 512], bf, tag="pt")
                            nc.scalar.activation(pt[:], sp[:], AF.Exp,
                                                 scale=1.0 / np.sqrt(HD))
                            if mi is not None:
                                nc.vector.tensor_mul(pt[:], pt[:], mt_sb[:, mi, :])
                            nc.tensor.matmul(
                                otps[:],
                                lhsT=vone_sb[:, kt, 65 * kv:65 * kv + 65],
                                rhs=pt[:],
                                start=(idx == 0), stop=(idx == len(kts) - 1))
                        # normalize: rowsum is otps row 64 (per-head halves)
                        ri = rip.tile([1, 512], f32, tag="ri")
                        nc.vector.reciprocal(ri[:], otps[64:65, :])
                        rib = rip.tile([1, 512], bf, tag="rib")
                        nc.vector.tensor_copy(rib[:], ri[:])
                        bcps = pm.tile([128, 512], f32, tag="misc")
                        nc.tensor.matmul(bcps[0:64, :], lhsT=ones_sb[:],
                                         rhs=rib[:], start=True, stop=True)
                        bcs = bcp.tile([64, 512], bf, tag="bc")
                        nc.vector.tensor_copy(bcs[:], bcps[0:64, :])
                        qo = q0 - s0
                        nc.vector.tensor_mul(ot_t[0:64, pair, qo:qo + 256],
                                             otps[0:64, 0:256], bcs[:, 0:256])
                        no_t = bcp.tile([64, 256], bf, tag="no")
                        nc.vector.tensor_mul(no_t[:], otps[0:64, 256:512],
                                             bcs[:, 256:512])
                        nc.vector.tensor_copy(ot_t[64:128, pair, qo:qo + 256],
                                              no_t[:])

                # ---- output projection for this stripe ----
                for j in range(4):
                    for nb in range(4):
                        wp = pm.tile([128, 512], f32, tag="misc")
                        for pr in range(4):
                            nc.tensor.matmul(wp[:],
                                             lhsT=ot_t[:, pr, j * 128:(j + 1) * 128],
                                             rhs=wo_sb[:, pr, nb * 512:(nb + 1) * 512],
                                             start=(pr == 0), stop=(pr == 3))
                        wsb = wsp.tile([128, 512], f32, tag="wsb")
                        nc.vector.tensor_copy(wsb[:], wp[:])
                        nc.sync.dma_start(
                            out.ap()[s0 + j * 128:s0 + (j + 1) * 128,
                                     nb * 512:(nb + 1) * 512],
                            wsb[:])
    nc.compile()
    return nc


def kernel(x, wq, wk, wv, wo, freqs, mask, start_pos):
    sys.path.insert(0, "/opt/trn_rl_repo")
    from concourse.bass_utils import run_bass_kernel_spmd

    x = np.asarray(x, dtype=np.float32)
    per_core, sched, U = _host_prepare(
        x, np.asarray(wq, np.float32), np.asarray(wk, np.float32),
        np.asarray(wv, np.float32), np.asarray(wo, np.float32),
        np.asarray(freqs, np.float32), np.asarray(mask, np.float32))

    nc = _build_program(sched, U)

    trace = bool(int(os.environ.get("BASSKERNEL_TRACE", "0")))
    if trace and "antenv.axon_hooks" not in sys.modules:
        # profile-hook shim (the trimmed antenv package lacks axon_hooks)
        try:
            import types

            if "/root/.axon_site" not in sys.path:
                sys.path.insert(0, "/root/.axon_site")
            from trn_agent_boot.trn_boot import _ntff_profile_via_ctypes

            _hook = _ntff_profile_via_ctypes("/opt/axon/libaxon_pjrt.so")
            _mod = types.ModuleType("antenv.axon_hooks")
            _mod.get_axon_ntff_profile_hook = lambda: _hook
            _mod.set_axon_ntff_profile_hook = lambda h: None
            sys.modules["antenv.axon_hooks"] = _mod
        except Exception:
            trace = False
    res = run_bass_kernel_spmd(nc, per_core, core_ids=list(range(NCORES)),
                               trace=trace)
    if trace:
        kernel._last_exec_time_ns = res.exec_time_ns
        kernel._last_profile = res.profile_json
    full = np.empty((B, S, D), np.float32)
    for b in range(B):
        acc = res.results[4 * b]["out"].astype(np.float64)
        for g in range(1, 4):
            acc += res.results[4 * b + g]["out"].astype(np.float64)
        full[b] = acc.astype(np.float32)
    return full
